# revision 4
# baseline (speedup 1.0000x reference)
"""KitNET anomaly-detection ensemble (25 tiny tied-weight autoencoders) on 8 Trainium2 cores.

Strategy (v2 — block-diagonal feature-permuted formulation):
  - Data-parallel over batch: each of the 8 cores processes B/8 = 16384 samples.
  - The feature gather x[:, idx] AND the transpose to feature-major are done on
    the host: x is shipped as bf16, feature-major, in AE-grouped (permuted)
    order, 4 chunks of 7/7/7/4 AEs. Each chunk's 112 (64) features fit in 128
    partitions together with a constant ones-row used to fold the biases into
    the matmuls.
  - Per 128-partition chunk the encode / decode / group-sum matrices are block
    diagonal, so encode+decode are 4 matmuls each per 512-sample tile (instead
    of dense 400x300), with hb/vb folded in via the ones-row (encode) and a
    sigmoid(0)=0.5 constant row (decode, coefficient 2*vb).
  - err = x - rec and err^2 run on the vector engine (bf16, 2x mode); per-AE
    mean-squared-errors are a 32-wide G matmul (entries 1/16) accumulated for
    4 batch tiles into one PSUM bank via tile_position column offsets.
  - sqrt(S + eps) is phase-split to the end (one ACT table switch); the final
    sum over the 25 AEs is a ones-matrix fp32 matmul.
"""

import sys

for _p in ("/opt/trn_rl_repo", "/opt/pypackages"):
    if _p not in sys.path:
        sys.path.append(_p)

import numpy as np

B = 131072
F = 400          # features
N_AE = 25
KF = 16          # features per AE
H = 12           # hidden per AE
EPS = 1e-6
N_CORES = 8
BC = B // N_CORES    # 16384 samples per core
NB = 512             # batch tile (matmul moving free dim)
NT = BC // NB        # 32 tiles per core
NG = NT // 4         # 8 groups of 4 tiles sharing one PSUM S bank

NAE_C = (7, 7, 7, 4)                      # AEs per 128-partition chunk
CR = tuple(1 + KF * n for n in NAE_C)     # contraction rows (1 ones-row + feats)
HR = tuple(1 + H * n for n in NAE_C)      # hidden rows (1 const row + hiddens)

_NC_CACHE = {}


def _build_nc():
    import concourse.tile as tile
    from concourse import bacc, mybir

    f32 = mybir.dt.float32
    bf16 = mybir.dt.bfloat16
    AF = mybir.ActivationFunctionType

    nc = bacc.Bacc()

    x_d = nc.declare_dram_parameter("x", [4, 113, BC], bf16, isOutput=False)
    wenc_d = nc.declare_dram_parameter("wenc", [4, 113, 85], bf16, isOutput=False)
    wdec_d = nc.declare_dram_parameter("wdec", [4, 85, 113], bf16, isOutput=False)
    g_d = nc.declare_dram_parameter("gmat", [4, 113, 32], bf16, isOutput=False)
    ones_d = nc.declare_dram_parameter("ones4", [128, 4], f32, isOutput=False)
    y_d = nc.declare_dram_parameter("y", [BC], f32, isOutput=True)

    with tile.TileContext(nc) as tc:
        with (
            tc.tile_pool(name="singles", bufs=1) as singles,
            tc.tile_pool(name="xt", bufs=2) as xt_p,
            tc.tile_pool(name="ht", bufs=2) as ht_p,
            tc.tile_pool(name="rec", bufs=2) as rec_p,
            tc.tile_pool(name="mmp", bufs=2, space="PSUM") as mmp,
            tc.tile_pool(name="sp", bufs=2, space="PSUM") as sp_p,
            tc.tile_pool(name="yp", bufs=2, space="PSUM") as yp_p,
        ):
            # --- constants ---
            wenc_sb = singles.tile([113, 4, 85], bf16)
            nc.sync.dma_start(
                out=wenc_sb, in_=wenc_d[:, :, :].rearrange("c p n -> p c n")
            )
            wdec_sb = singles.tile([85, 4, 113], bf16)
            nc.sync.dma_start(
                out=wdec_sb, in_=wdec_d[:, :, :].rearrange("c p n -> p c n")
            )
            g_sb = singles.tile([113, 4, 32], bf16)
            nc.sync.dma_start(out=g_sb, in_=g_d[:, :, :].rearrange("c p n -> p c n"))
            ones_sb = singles.tile([128, 4], f32)
            nc.sync.dma_start(out=ones_sb, in_=ones_d[:, :])
            eps_sb = singles.tile([128, 1], f32)
            nc.gpsimd.memset(eps_sb, EPS)
            # per-(AE, group) mean-squared errors for the whole core:
            # sall[32*(t%4) + 8*c + a', t//4, i]
            sall = singles.tile([128, NG, NB], f32)
            ybuf = singles.tile([4, NG, NB], f32)

            for tp in range(NT // 2):
                # ---- load 2 tiles = 1024 samples, feature-major bf16
                xt = xt_p.tile([113, 4, 2, NB], bf16, tag="xt")
                nc.sync.dma_start(
                    out=xt,
                    in_=x_d[:, :, tp * 2 * NB:(tp + 1) * 2 * NB].rearrange(
                        "c p (u i) -> p c u i", u=2
                    ),
                )
                for u in range(2):
                    t = 2 * tp + u
                    g = t % 4

                    # ---- encode: ht = sigmoid(Wenc^T @ xt)  (hb via ones-row)
                    ht = ht_p.tile([128, 4, NB], bf16, tag="ht")
                    for half in range(2):
                        pe_ = mmp.tile([128, 2, NB], f32, tag="mm")
                        for cc in range(2):
                            c = 2 * half + cc
                            nc.tensor.matmul(
                                pe_[0:85, cc, :],
                                lhsT=wenc_sb[0:CR[c], c, :],
                                rhs=xt[0:CR[c], c, u, :],
                                start=True,
                                stop=True,
                            )
                        nc.scalar.activation(
                            out=ht[:, 2 * half:2 * half + 2, :],
                            in_=pe_,
                            func=AF.Sigmoid,
                        )

                    # ---- decode: rec = sigmoid(Wdec^T @ ht)  (vb via 0.5-row)
                    rec = rec_p.tile([128, 4, NB], bf16, tag="rec")
                    for half in range(2):
                        pd = mmp.tile([128, 2, NB], f32, tag="mm")
                        for cc in range(2):
                            c = 2 * half + cc
                            nc.tensor.matmul(
                                pd[0:113, cc, :],
                                lhsT=wdec_sb[0:HR[c], c, :],
                                rhs=ht[0:HR[c], c, :],
                                start=True,
                                stop=True,
                            )
                        nc.scalar.activation(
                            out=rec[:, 2 * half:2 * half + 2, :],
                            in_=pd,
                            func=AF.Sigmoid,
                        )

                    # ---- err^2 in place (DVE, bf16 2x)
                    nc.vector.tensor_sub(
                        rec[0:113], xt[:, :, u, :], rec[0:113]
                    )
                    nc.vector.tensor_mul(rec[0:113], rec[0:113], rec[0:113])

                    # ---- per-AE mean: S[32g + 8c + a'] += G^T @ err2
                    if g == 0:
                        S = sp_p.tile([128, NB], f32, tag="s")
                    for c in range(4):
                        nc.tensor.matmul(
                            S[32 * g:32 * (g + 1), :],
                            lhsT=g_sb[0:CR[c], c, :],
                            rhs=rec[0:CR[c], c, :],
                            start=(c == 0),
                            stop=(c == 3),
                            tile_position=(0, 32 * g),
                        )
                    if g == 3:
                        nc.vector.tensor_copy(out=sall[:, t // 4, :], in_=S)

            # ---- phase B: rmse = sqrt(S + eps); y = sum over AEs
            nc.scalar.activation(
                out=sall, in_=sall, func=AF.Sqrt, bias=eps_sb, scale=1.0
            )
            for j in range(NG):
                py = yp_p.tile([4, NB], f32, tag="y")
                nc.tensor.matmul(
                    py, lhsT=ones_sb, rhs=sall[:, j, :], start=True, stop=True
                )
                if j % 2 == 0:
                    nc.vector.tensor_copy(out=ybuf[:, j, :], in_=py)
                else:
                    nc.scalar.copy(out=ybuf[:, j, :], in_=py)
            # y[b], b = t*NB + i, t = 4j + g  ->  y view [g, j, i]
            y_ap = y_d[:].rearrange("(j g i) -> g j i", g=4, i=NB)
            nc.sync.dma_start(out=y_ap, in_=ybuf)

    nc.compile()
    return nc


def _host_mats(W, hb, vb, idx):
    import ml_dtypes

    bf16 = ml_dtypes.bfloat16
    W = np.asarray(W, np.float32)
    hb = np.asarray(hb, np.float32)
    vb = np.asarray(vb, np.float32)
    idx = np.asarray(idx)

    wenc = np.zeros((4, 113, 85), np.float32)
    wdec = np.zeros((4, 85, 113), np.float32)
    gmat = np.zeros((4, 113, 32), np.float32)
    ones4 = np.zeros((128, 4), np.float32)
    for c in range(4):
        for ap in range(NAE_C[c]):
            a = 7 * c + ap
            fr = 1 + KF * ap          # feature row base (within chunk)
            hr = 1 + H * ap           # hidden row/col base
            wenc[c, 0, hr:hr + H] = hb[a, :]
            wenc[c, fr:fr + KF, hr:hr + H] = W[a, :, :]
            wdec[c, 0, fr:fr + KF] = 2.0 * vb[a, :]
            wdec[c, hr:hr + H, fr:fr + KF] = W[a, :, :].T
            gmat[c, fr:fr + KF, 8 * c + ap] = 1.0 / KF
            ones4[np.arange(4) * 32 + 8 * c + ap, np.arange(4)] = 1.0

    return {
        "wenc": np.ascontiguousarray(wenc.astype(bf16)),
        "wdec": np.ascontiguousarray(wdec.astype(bf16)),
        "gmat": np.ascontiguousarray(gmat.astype(bf16)),
        "ones4": ones4,
    }


def _host_x(x, idx):
    """Full x [B, 400] f32 -> per-core [4, 113, BC] bf16, feature-major,
    AE-grouped order, ones row at each chunk's row 0."""
    import ml_dtypes

    bf16 = ml_dtypes.bfloat16
    perm = np.asarray(idx).reshape(-1)          # AE-major feature order
    xg = np.asarray(x, np.float32)[:, perm]     # [B, 400]
    xt = xg.T.astype(bf16)                      # [400, B]
    xfull = np.zeros((4, 113, B), bf16)
    xfull[:, 0, :] = bf16(1.0)
    for c in range(4):
        w = KF * NAE_C[c]
        xfull[c, 1:1 + w, :] = xt[112 * c:112 * c + w]
    return xfull


def _get_nc():
    if "nc" not in _NC_CACHE:
        _NC_CACHE["nc"] = _build_nc()
    return _NC_CACHE["nc"]


def _run(x, W, hb, vb, idx, trace=False):
    from concourse.bass_utils import run_bass_kernel_spmd

    consts = _host_mats(W, hb, vb, idx)
    xfull = _host_x(x, idx)
    in_maps = [
        {
            "x": np.ascontiguousarray(xfull[:, :, c * BC:(c + 1) * BC]),
            **consts,
        }
        for c in range(N_CORES)
    ]
    nc = _get_nc()
    res = run_bass_kernel_spmd(nc, in_maps, list(range(N_CORES)), trace=trace)
    y = np.concatenate([res.results[c]["y"] for c in range(N_CORES)])
    return y, res


def kernel(x, W, hb, vb, idx):
    y, _ = _run(x, W, hb, vb, idx)
    return y


# revision 10
# speedup vs baseline: 1.0618x; 1.0618x over previous
"""KitNET anomaly-detection ensemble (25 tiny tied-weight autoencoders) on 8 Trainium2 cores.

Strategy (v2 — block-diagonal feature-permuted formulation):
  - Data-parallel over batch: each of the 8 cores processes B/8 = 16384 samples.
  - The feature gather x[:, idx] AND the transpose to feature-major are done on
    the host: x is shipped as bf16, feature-major, in AE-grouped (permuted)
    order, 4 chunks of 7/7/7/4 AEs. Each chunk's 112 (64) features fit in 128
    partitions together with a constant ones-row used to fold the biases into
    the matmuls.
  - Per 128-partition chunk the encode / decode / group-sum matrices are block
    diagonal, so encode+decode are 4 matmuls each per 512-sample tile (instead
    of dense 400x300), with hb/vb folded in via the ones-row (encode) and a
    sigmoid(0)=0.5 constant row (decode, coefficient 2*vb).
  - err = x - rec and err^2 run on the vector engine (bf16, 2x mode); per-AE
    mean-squared-errors are a 32-wide G matmul (entries 1/16) accumulated for
    4 batch tiles into one PSUM bank via tile_position column offsets.
  - sqrt(S + eps) is phase-split to the end (one ACT table switch); the final
    sum over the 25 AEs is a ones-matrix fp32 matmul.
"""

import sys

for _p in ("/opt/trn_rl_repo", "/opt/pypackages"):
    if _p not in sys.path:
        sys.path.append(_p)

import numpy as np

B = 131072
F = 400          # features
N_AE = 25
KF = 16          # features per AE
H = 12           # hidden per AE
EPS = 1e-6
N_CORES = 8
BC = B // N_CORES    # 16384 samples per core
NB = 512             # batch tile (matmul moving free dim)
NT = BC // NB        # 32 tiles per core
NG = NT // 4         # 8 groups of 4 tiles sharing one PSUM S bank

NAE_C = (7, 7, 7, 4)                      # AEs per 128-partition chunk
CR = tuple(1 + KF * n for n in NAE_C)     # contraction rows (1 ones-row + feats)
HR = tuple(1 + H * n for n in NAE_C)      # hidden rows (1 const row + hiddens)

_NC_CACHE = {}


def _build_nc():
    import concourse.tile as tile
    from concourse import bacc, mybir

    f32 = mybir.dt.float32
    bf16 = mybir.dt.bfloat16
    AF = mybir.ActivationFunctionType

    nc = bacc.Bacc()

    x_d = nc.declare_dram_parameter(
        "x", [NT // 2, 113, 2, 4, NB], bf16, isOutput=False
    )
    wenc_d = nc.declare_dram_parameter("wenc", [4, 113, 85], bf16, isOutput=False)
    wdec_d = nc.declare_dram_parameter("wdec", [4, 85, 113], bf16, isOutput=False)
    g_d = nc.declare_dram_parameter("gmat", [4, 113, 32], bf16, isOutput=False)
    ones_d = nc.declare_dram_parameter("ones4", [128, 4], f32, isOutput=False)
    y_d = nc.declare_dram_parameter("y", [BC], f32, isOutput=True)

    with tile.TileContext(nc) as tc:
        with (
            tc.tile_pool(name="singles", bufs=1) as singles,
            tc.tile_pool(name="xt", bufs=2) as xt_p,
            tc.tile_pool(name="ht", bufs=2) as ht_p,
            tc.tile_pool(name="rec", bufs=2) as rec_p,
            tc.tile_pool(name="mmp", bufs=2, space="PSUM") as mmp,
            tc.tile_pool(name="sp", bufs=2, space="PSUM") as sp_p,
            tc.tile_pool(name="yp", bufs=2, space="PSUM") as yp_p,
        ):
            # --- constants ---
            wenc_sb = singles.tile([113, 4, 85], bf16)
            nc.sync.dma_start(
                out=wenc_sb, in_=wenc_d[:, :, :].rearrange("c p n -> p c n")
            )
            wdec_sb = singles.tile([85, 4, 113], bf16)
            nc.sync.dma_start(
                out=wdec_sb, in_=wdec_d[:, :, :].rearrange("c p n -> p c n")
            )
            g_sb = singles.tile([113, 4, 32], bf16)
            nc.sync.dma_start(out=g_sb, in_=g_d[:, :, :].rearrange("c p n -> p c n"))
            ones_sb = singles.tile([128, 4], f32)
            nc.sync.dma_start(out=ones_sb, in_=ones_d[:, :])
            eps_sb = singles.tile([128, 1], f32)
            nc.gpsimd.memset(eps_sb, EPS)
            # per-(AE, group) mean-squared errors for the whole core:
            # sall[32*(t%4) + 8*c + a', t//4, i]
            sall = singles.tile([128, NG, NB], f32)
            ybuf = singles.tile([4, NG, NB], f32)

            for tp in range(NT // 2):
                # ---- load 2 tiles = 1024 samples, feature-major bf16
                # (one contiguous 8 KB line per partition in DRAM)
                xt = xt_p.tile([113, 2, 4, NB], bf16, tag="xt")
                nc.sync.dma_start(out=xt, in_=x_d[tp])
                for u in range(2):
                    t = 2 * tp + u
                    g = t % 4

                    # ---- encode: ht = sigmoid(Wenc^T @ xt)  (hb via ones-row)
                    ht = ht_p.tile([128, 4, NB], bf16, tag="ht")
                    for half in range(2):
                        pe_ = mmp.tile([128, 2, NB], f32, tag="mm")
                        for cc in range(2):
                            c = 2 * half + cc
                            nc.tensor.matmul(
                                pe_[0:85, cc, :],
                                lhsT=wenc_sb[0:CR[c], c, :],
                                rhs=xt[0:CR[c], u, c, :],
                                start=True,
                                stop=True,
                            )
                        nc.scalar.activation(
                            out=ht[:, 2 * half:2 * half + 2, :],
                            in_=pe_,
                            func=AF.Sigmoid,
                        )

                    # ---- decode: rec = sigmoid(Wdec^T @ ht)  (vb via 0.5-row)
                    rec = rec_p.tile([128, 4, NB], bf16, tag="rec")
                    for half in range(2):
                        pd = mmp.tile([128, 2, NB], f32, tag="mm")
                        for cc in range(2):
                            c = 2 * half + cc
                            nc.tensor.matmul(
                                pd[0:113, cc, :],
                                lhsT=wdec_sb[0:HR[c], c, :],
                                rhs=ht[0:HR[c], c, :],
                                start=True,
                                stop=True,
                            )
                        nc.scalar.activation(
                            out=rec[:, 2 * half:2 * half + 2, :],
                            in_=pd,
                            func=AF.Sigmoid,
                        )

                    # ---- err^2 in place (DVE, bf16 2x)
                    nc.vector.tensor_sub(
                        rec[0:113], xt[:, u, :, :], rec[0:113]
                    )
                    nc.vector.tensor_mul(rec[0:113], rec[0:113], rec[0:113])

                    # ---- per-AE mean: S[32g + 8c + a'] += G^T @ err2
                    if g == 0:
                        S = sp_p.tile([128, NB], f32, tag="s")
                    for c in range(4):
                        nc.tensor.matmul(
                            S[32 * g:32 * (g + 1), :],
                            lhsT=g_sb[0:CR[c], c, :],
                            rhs=rec[0:CR[c], c, :],
                            start=(c == 0),
                            stop=(c == 3),
                            tile_position=(0, 32 * g),
                        )
                    if g == 3:
                        nc.vector.tensor_copy(out=sall[:, t // 4, :], in_=S)

            # ---- phase B: rmse = sqrt(S + eps); y = sum over AEs
            nc.scalar.activation(
                out=sall, in_=sall, func=AF.Sqrt, bias=eps_sb, scale=1.0
            )
            for j in range(NG):
                py = yp_p.tile([4, NB], f32, tag="y")
                nc.tensor.matmul(
                    py, lhsT=ones_sb, rhs=sall[:, j, :], start=True, stop=True
                )
                if j % 2 == 0:
                    nc.vector.tensor_copy(out=ybuf[:, j, :], in_=py)
                else:
                    nc.scalar.copy(out=ybuf[:, j, :], in_=py)
            # y[b], b = t*NB + i, t = 4j + g  ->  y view [g, j, i]
            y_ap = y_d[:].rearrange("(j g i) -> g j i", g=4, i=NB)
            nc.sync.dma_start(out=y_ap, in_=ybuf)

    nc.compile()
    return nc


def _host_mats(W, hb, vb, idx):
    import ml_dtypes

    bf16 = ml_dtypes.bfloat16
    W = np.asarray(W, np.float32)
    hb = np.asarray(hb, np.float32)
    vb = np.asarray(vb, np.float32)
    idx = np.asarray(idx)

    wenc = np.zeros((4, 113, 85), np.float32)
    wdec = np.zeros((4, 85, 113), np.float32)
    gmat = np.zeros((4, 113, 32), np.float32)
    ones4 = np.zeros((128, 4), np.float32)
    for c in range(4):
        for ap in range(NAE_C[c]):
            a = 7 * c + ap
            fr = 1 + KF * ap          # feature row base (within chunk)
            hr = 1 + H * ap           # hidden row/col base
            wenc[c, 0, hr:hr + H] = hb[a, :]
            wenc[c, fr:fr + KF, hr:hr + H] = W[a, :, :]
            wdec[c, 0, fr:fr + KF] = 2.0 * vb[a, :]
            wdec[c, hr:hr + H, fr:fr + KF] = W[a, :, :].T
            gmat[c, fr:fr + KF, 8 * c + ap] = 1.0 / KF
            ones4[np.arange(4) * 32 + 8 * c + ap, np.arange(4)] = 1.0

    return {
        "wenc": np.ascontiguousarray(wenc.astype(bf16)),
        "wdec": np.ascontiguousarray(wdec.astype(bf16)),
        "gmat": np.ascontiguousarray(gmat.astype(bf16)),
        "ones4": ones4,
    }


def _host_x(x, idx):
    """Full x [B, 400] f32 -> per-core [NT//2, 113, 2, 4, NB] bf16,
    feature-major, AE-grouped order, ones row at each chunk's row 0.
    DRAM layout gives each 2-tile load one contiguous 8 KB line per
    partition."""
    import ml_dtypes

    bf16 = ml_dtypes.bfloat16
    perm = np.asarray(idx).reshape(-1)          # AE-major feature order
    xg = np.asarray(x, np.float32)[:, perm]     # [B, 400] gather
    xt = xg.T                                   # [400, B] view
    out = []
    for core in range(N_CORES):
        xc = np.zeros((NT // 2, 113, 2, 4, NB), np.float32)
        xc[:, 0, :, :, :] = 1.0
        sl = xt[:, core * BC:(core + 1) * BC]   # [400, BC] view
        slr = sl.reshape(400, NT // 2, 2, NB)   # [f, tp, u, i]
        for c in range(4):
            w = KF * NAE_C[c]
            # [w, tp, u, i] -> [tp, w, u, i]
            xc[:, 1:1 + w, :, c, :] = slr[112 * c:112 * c + w].transpose(
                1, 0, 2, 3
            )
        out.append(xc.astype(bf16))
    return out


def _get_nc():
    if "nc" not in _NC_CACHE:
        _NC_CACHE["nc"] = _build_nc()
    return _NC_CACHE["nc"]


def _run(x, W, hb, vb, idx, trace=False):
    from concourse.bass_utils import run_bass_kernel_spmd

    consts = _host_mats(W, hb, vb, idx)
    xcores = _host_x(x, idx)
    in_maps = [{"x": xcores[c], **consts} for c in range(N_CORES)]
    nc = _get_nc()
    res = run_bass_kernel_spmd(nc, in_maps, list(range(N_CORES)), trace=trace)
    y = np.concatenate([res.results[c]["y"] for c in range(N_CORES)])
    return y, res


def kernel(x, W, hb, vb, idx):
    y, _ = _run(x, W, hb, vb, idx)
    return y
